# revision 26
# baseline (speedup 1.0000x reference)
"""Trainium2 Bass kernel for nn_Actor (topk_masking).

Reference semantics:
    s    = einsum('ol,bld->bod', W, state)[:, 0, :]        # (B, D) weighted sum over L
    a0   = softmax(s, axis=-1)
    loop T-1 times: zero the argmax entry, renormalize; stack all T states
    out  = (B, T, D)

Identity used: out[t] = (e with top-t entries zeroed) * C_t with
e = exp(w * sum_l x_l), C_t = 1/D_t, D_t = suffix sum of top-24 values + R.
Row t is built by thresholding:  out[t] = (e < v_t) * e * C_t  (v_t = t-th
largest of e), so the T rows are independent ops, not a serial chain.

The L-sum runs mostly on the otherwise-idle TensorE as an *identity-matmul
accumulator*: stationary = I_128 (fp32, exact), moving = the natural
b-major mega slice [128, 500], PSUM accumulates s[128, 500] over the
l-matmuls — no data reshaping, no cross-partition moves, loads stay in the
fast DMA shape.  fp32 matmul cost is per output row and K-independent.
Hard-won constraints baked in (probed on HW):
  - A PSUM accumulation group must live on a single-bank [128, 500] tile;
    two groups on a two-bank tile corrupt, and *interleaving* two groups'
    matmuls crashes the device (NRT_EXEC_UNIT_UNRECOVERABLE).  So the two
    500-wide d-halves are processed as fully sequential groups, with megas
    loaded per d-half ([128, 5, 500], ~348 GB/s vs 403 contiguous).
  - PE covers l=0..31 plus the DVE partial (33 matmuls per group); VectorE
    sums l=32..49 and feeds its partial in as the last matmul.  All-PE
    would be ~400 us (985 ns/accumulating-matmul); the hybrid is ~260.
  - GpSimd tensor ops are unused: they poison concurrent DVE throughput
    (measured 2-10x slowdown).
  - Output rows are staged fp16 (tolerance 2e-2 >> fp16's ~1e-4) and
    widened to f32 on host; halves HBM write traffic.

Sharding: pure data parallel over the batch dim across 8 NeuronCores.
"""

import numpy as np

from concourse import bacc, bass, mybir
from concourse import tile as tile_mod
from concourse.bass_utils import run_bass_kernel_spmd

F32 = mybir.dt.float32
F16 = mybir.dt.float16
AF = mybir.ActivationFunctionType
ALU = mybir.AluOpType

# Problem constants (hardcoded per harness contract)
B_FULL = 4096
L = 50
D = 1000
T = 20
N_CORES = 8
B_CORE = B_FULL // N_CORES  # 512
P = 128                     # partitions per tile
N_TILES = B_CORE // P       # 4
DH = D // 2                 # 500 = one PSUM bank of f32

L_PE = 28                   # l = 0..27 summed on PE (uniform path)
MEGA_L = 5
N_MEGA = L // MEGA_L        # 10 megas per d-half
ROWS_PER_STORE = 5          # output rows per store DMA (1.25 MB fp16)


def build_graph(b_core=B_CORE, general=False):
    """Uniform path (general=False): PE sums l=0..31 via an identity
    stationary, DVE sums l=32..49; the uniform weight is folded into the
    exp scale.  General path: all 50 l's go through PE with per-l diag(w_l)
    stationaries (wmat holds 50 of them)."""
    n_tiles = b_core // P
    n_wm = L if general else 1
    nc = bacc.Bacc("TRN2")
    # state pre-split into d-halves on host: contiguous [b, L, 500] each,
    # so the per-half mega loads are contiguous 10 KB/partition lines
    # (the strided [*, 5, 500] view of a full [b, L, 1000] array runs at
    # 348 GB/s vs ~403 contiguous)
    half_ext = [
        nc.declare_dram_parameter(f"state{h}", [b_core, L, DH], F32, isOutput=False)
        for h in range(2)
    ]
    wmat_ext = nc.declare_dram_parameter("wmat", [P, n_wm, P], F32, isOutput=False)
    out_ext = nc.declare_dram_parameter("out", [b_core, T, D], F16, isOutput=True)

    with tile_mod.TileContext(nc) as tc:
        with (
            tc.tile_pool(name="mega", bufs=13) as mega_pool,
            tc.tile_pool(name="w1p", bufs=2) as w1_pool,
            tc.tile_pool(name="part", bufs=4) as part_pool,
            tc.tile_pool(name="epool", bufs=2) as e_pool,
            tc.tile_pool(name="tmp", bufs=4) as tmp_pool,
            tc.tile_pool(name="rows", bufs=2) as row_pool,
            tc.tile_pool(name="small", bufs=2) as small_pool,
            tc.tile_pool(name="wm", bufs=1) as w_pool,
            tc.tile_pool(name="ps", bufs=4, space="PSUM") as ps_pool,
        ):
            wm = w_pool.tile([P, n_wm, P], F32, tag="wm")
            nc.sync.dma_start(wm[:], wmat_ext[:])

            for bt in range(n_tiles):
                b0 = bt * P
                e0 = e_pool.tile([P, D], F32, tag="e")

                for h in range(2):
                    d0 = h * DH
                    megas = [None] * N_MEGA

                    def load_mega(m):
                        M = mega_pool.tile([P, MEGA_L, DH], F32, tag="mega")
                        nc.sync.dma_start(
                            M[:],
                            half_ext[h][
                                b0 : b0 + P,
                                m * MEGA_L : (m + 1) * MEGA_L,
                                :,
                            ],
                        )
                        megas[m] = M

                    # PE megas early (they release slots fastest); DVE
                    # chain megas (7,8,9) soon after so its chain overlaps
                    if general:
                        order = list(range(N_MEGA))
                    else:
                        order = [0, 1, 7, 8, 9, 2, 3, 4, 5, 6]
                    for m in order:
                        load_mega(m)

                    pt = ps_pool.tile([P, DH], F32, tag="p")

                    if general:
                        # all 50 l's through PE with diag(w_l) stationaries
                        for l in range(L):
                            nc.tensor.matmul(
                                pt[:],
                                wm[:, l, :],
                                megas[l // MEGA_L][:, l % MEGA_L, :],
                                start=(l == 0),
                                stop=(l == L - 1),
                            )
                    else:
                        # ---- DVE: partial = sum of l=28..49 ----
                        w1 = w1_pool.tile([P, MEGA_L, DH], F32, tag="w1")
                        nc.vector.tensor_tensor(
                            w1[:], megas[7][:], megas[8][:], ALU.add
                        )
                        nc.vector.tensor_tensor(
                            w1[:], w1[:], megas[9][:], ALU.add
                        )
                        partial = part_pool.tile([P, DH], F32, tag="part")
                        nc.vector.tensor_tensor(
                            partial[:], w1[:, 0, :], w1[:, 1, :], ALU.add
                        )
                        for j in range(2, MEGA_L):
                            nc.vector.tensor_tensor(
                                partial[:], partial[:], w1[:, j, :], ALU.add
                            )
                        for j in range(MEGA_L):
                            nc.vector.tensor_tensor(
                                partial[:], partial[:], megas[6][:, j, :],
                                ALU.add,
                            )
                        for j in range(L_PE % MEGA_L, MEGA_L):
                            nc.vector.tensor_tensor(
                                partial[:], partial[:],
                                megas[L_PE // MEGA_L][:, j, :], ALU.add,
                            )

                        # ---- PE: identity-accumulate l=0..L_PE-1 ----
                        for l in range(L_PE):
                            nc.tensor.matmul(
                                pt[:],
                                wm[:, 0, :],
                                megas[l // MEGA_L][:, l % MEGA_L, :],
                                start=(l == 0),
                                stop=(l == L_PE - 1),
                            )

                    # ---- e-half: exp(s_PE) * exp(s_DVE) ----
                    # (keeps the PE group independent of the DVE partial;
                    # exp of each part, then one DVE multiply)
                    nc.scalar.activation(
                        e0[:, d0 : d0 + DH], pt[:], AF.Exp, bias=0.0,
                        scale=1.0,
                    )
                    if not general:
                        pd = part_pool.tile([P, DH], F32, tag="pexp")
                        nc.scalar.activation(
                            pd[:], partial[:], AF.Exp, bias=0.0, scale=1.0
                        )
                        nc.vector.tensor_tensor(
                            e0[:, d0 : d0 + DH], e0[:, d0 : d0 + DH],
                            pd[:], ALU.mult,
                        )

                # ---- top-24 values + R = sum of the rest ----
                st = small_pool.tile([P, 104], F32, tag="stats")
                v_pad = st[:, 0:31]
                suf = st[:, 32:56]
                Dt = st[:, 56:76]
                Ct = st[:, 76:96]
                R = st[:, 96:97]
                nc.vector.memset(v_pad[:, 0:7], -1.0)
                va = v_pad[:, 7:15]
                vb = v_pad[:, 15:23]
                vc = v_pad[:, 23:31]
                u = tmp_pool.tile([P, D], F32, tag="tmp")
                nc.vector.max(va, e0[:])
                nc.vector.match_replace(u[:], va, e0[:], 0.0)
                nc.vector.max(vb, u[:])
                nc.vector.match_replace(u[:], vb, u[:], 0.0)
                nc.vector.max(vc, u[:])
                nc.vector.match_replace(u[:], vc, u[:], 0.0)
                nc.vector.tensor_reduce(
                    R, u[:], axis=mybir.AxisListType.X, op=ALU.add
                )

                # ---- D_t = suffix_sum(v_{t+1..24}) + R ;  C = 1/D ----
                v_rev = v_pad[:, 30:6:-1]
                nc.vector.tensor_tensor_scan(
                    suf, v_rev, v_rev, 0.0, ALU.add, ALU.bypass
                )
                nc.vector.tensor_scalar(
                    Dt, suf[:, 23:3:-1], R, None, ALU.add
                )
                nc.vector.reciprocal(Ct, Dt)

                # ---- rows: t=0 plain; t>=1 threshold-masked, all indep ----
                rowgs = {}
                for t in range(T):
                    g = t // ROWS_PER_STORE
                    j = t % ROWS_PER_STORE
                    if g not in rowgs:
                        rowgs[g] = row_pool.tile(
                            [P, ROWS_PER_STORE, D], F16, tag="rowg",
                            name=f"rowg_{bt}_{g}",
                        )
                    if t == 0:
                        src_row = e0
                    else:
                        src_row = tmp_pool.tile([P, D], F32, tag="tmp")
                        # (e0 < v_t) * e0 ; v_t = t-th largest = v_pad[6+t]
                        nc.vector.scalar_tensor_tensor(
                            src_row[:], e0[:], v_pad[:, 6 + t : 7 + t],
                            e0[:], ALU.is_lt, ALU.mult,
                        )
                    nc.scalar.activation(
                        rowgs[g][:, j, :], src_row[:], AF.Copy, bias=0.0,
                        scale=Ct[:, t : t + 1],
                    )
                    if j == ROWS_PER_STORE - 1:
                        # scalar-ring store: keeps the in-order sync queue
                        # free for the load stream (a store waits on its row
                        # copies and would stall the next tile's loads)
                        nc.scalar.dma_start(
                            out_ext[b0 : b0 + P, t - j : t + 1, :],
                            rowgs[g][:],
                        )

    nc.finalize()
    return nc


_GRAPH_CACHE = {}


def _get_graph(w):
    w = np.asarray(w, dtype=np.float32).reshape(-1)
    assert w.shape[0] == L
    if np.all(w == w[0]):
        # uniform: identity stationary, weight folded into the exp scale
        # (exp scale handled by scaling e? no — exp(scale*s): the scale is
        # baked as s is unweighted; but uniform_w == 1.0 for the harness.
        # For uniform_w != 1.0 we reuse the general path to stay exact.)
        if float(w[0]) == 1.0:
            key = "uniform"
            if key not in _GRAPH_CACHE:
                _GRAPH_CACHE[key] = build_graph(general=False)
            wmat = np.eye(P, dtype=np.float32).reshape(P, 1, P)
            return _GRAPH_CACHE[key], wmat
    key = "general"
    if key not in _GRAPH_CACHE:
        _GRAPH_CACHE[key] = build_graph(general=True)
    wmat = np.zeros((P, L, P), dtype=np.float32)
    for l in range(L):
        np.fill_diagonal(wmat[:, l, :], w[l])
    return _GRAPH_CACHE[key], wmat


def kernel(state, weight_matrix):
    state = np.ascontiguousarray(np.asarray(state, dtype=np.float32))
    w = np.asarray(weight_matrix, dtype=np.float32)
    assert state.shape == (B_FULL, L, D), state.shape

    nc, wmat = _get_graph(w)
    # pre-split the d-halves so device loads are contiguous (layout prep,
    # same class as the per-core sharding slices)
    state_lo = np.ascontiguousarray(state[:, :, :DH])
    state_hi = np.ascontiguousarray(state[:, :, DH:])
    in_maps = [
        {
            "state0": state_lo[i * B_CORE : (i + 1) * B_CORE],
            "state1": state_hi[i * B_CORE : (i + 1) * B_CORE],
            "wmat": wmat,
        }
        for i in range(N_CORES)
    ]
    res = run_bass_kernel_spmd(nc, in_maps, core_ids=list(range(N_CORES)))
    out = np.concatenate(
        [
            np.asarray(res.results[i]["out"], dtype=np.float32)
            for i in range(N_CORES)
        ],
        axis=0,
    )
    return out


# revision 29
# speedup vs baseline: 1.0369x; 1.0369x over previous
"""Trainium2 Bass kernel for nn_Actor (topk_masking).

Reference semantics:
    s    = einsum('ol,bld->bod', W, state)[:, 0, :]        # (B, D) weighted sum over L
    a0   = softmax(s, axis=-1)
    loop T-1 times: zero the argmax entry, renormalize; stack all T states
    out  = (B, T, D)

Identity used: out[t] = (e with top-t entries zeroed) * C_t with
e = exp(w * sum_l x_l), C_t = 1/D_t, D_t = suffix sum of top-24 values + R.
Row t is built by thresholding:  out[t] = (e < v_t) * e * C_t  (v_t = t-th
largest of e), so the T rows are independent ops, not a serial chain.

The L-sum runs mostly on the otherwise-idle TensorE as an *identity-matmul
accumulator*: stationary = I_128 (fp32, exact), moving = the natural
b-major mega slice [128, 500], PSUM accumulates s[128, 500] over the
l-matmuls — no data reshaping, no cross-partition moves, loads stay in the
fast DMA shape.  fp32 matmul cost is per output row and K-independent.
Hard-won constraints baked in (probed on HW):
  - A PSUM accumulation group must live on a single-bank [128, 500] tile;
    two groups on a two-bank tile corrupt, and *interleaving* two groups'
    matmuls crashes the device (NRT_EXEC_UNIT_UNRECOVERABLE).  So the two
    500-wide d-halves are processed as fully sequential groups, with megas
    loaded per d-half ([128, 5, 500], ~348 GB/s vs 403 contiguous).
  - PE covers l=0..31 plus the DVE partial (33 matmuls per group); VectorE
    sums l=32..49 and feeds its partial in as the last matmul.  All-PE
    would be ~400 us (985 ns/accumulating-matmul); the hybrid is ~260.
  - GpSimd tensor ops are unused: they poison concurrent DVE throughput
    (measured 2-10x slowdown).
  - Output rows are staged fp16 (tolerance 2e-2 >> fp16's ~1e-4) and
    widened to f32 on host; halves HBM write traffic.

Sharding: pure data parallel over the batch dim across 8 NeuronCores.
"""

import numpy as np

from concourse import bacc, bass, mybir
from concourse import tile as tile_mod
from concourse.bass_utils import run_bass_kernel_spmd

F32 = mybir.dt.float32
F16 = mybir.dt.float16
AF = mybir.ActivationFunctionType
ALU = mybir.AluOpType

# Problem constants (hardcoded per harness contract)
B_FULL = 4096
L = 50
D = 1000
T = 20
N_CORES = 8
B_CORE = B_FULL // N_CORES  # 512
P = 128                     # partitions per tile
N_TILES = B_CORE // P       # 4
DH = D // 2                 # 500 = one PSUM bank of f32

L_PE = 28                   # l = 0..27 summed on PE (uniform path)
MEGA_L = 5
N_MEGA = L // MEGA_L        # 10 megas per d-half
ROWS_PER_STORE = 5          # output rows per store DMA (1.25 MB fp16)


def build_graph(b_core=B_CORE, general=False):
    """Uniform path (general=False): PE sums l=0..31 via an identity
    stationary, DVE sums l=32..49; the uniform weight is folded into the
    exp scale.  General path: all 50 l's go through PE with per-l diag(w_l)
    stationaries (wmat holds 50 of them)."""
    n_tiles = b_core // P
    n_wm = L if general else 1
    nc = bacc.Bacc("TRN2")
    # state pre-split into d-halves on host: contiguous [b, L, 500] each,
    # so the per-half mega loads are contiguous 10 KB/partition lines
    # (the strided [*, 5, 500] view of a full [b, L, 1000] array runs at
    # 348 GB/s vs ~403 contiguous)
    half_ext = [
        nc.declare_dram_parameter(f"state{h}", [b_core, L, DH], F32, isOutput=False)
        for h in range(2)
    ]
    wmat_ext = nc.declare_dram_parameter("wmat", [P, n_wm, P], F32, isOutput=False)
    out_ext = nc.declare_dram_parameter("out", [b_core, T, D], F16, isOutput=True)

    with tile_mod.TileContext(nc) as tc:
        with (
            tc.tile_pool(name="mega", bufs=13) as mega_pool,
            tc.tile_pool(name="w1p", bufs=2) as w1_pool,
            tc.tile_pool(name="part", bufs=4) as part_pool,
            tc.tile_pool(name="epool", bufs=2) as e_pool,
            tc.tile_pool(name="tmp", bufs=4) as tmp_pool,
            tc.tile_pool(name="rows", bufs=2) as row_pool,
            tc.tile_pool(name="small", bufs=2) as small_pool,
            tc.tile_pool(name="wm", bufs=1) as w_pool,
            tc.tile_pool(name="ps", bufs=4, space="PSUM") as ps_pool,
        ):
            wm = w_pool.tile([P, n_wm, P], F32, tag="wm")
            nc.sync.dma_start(wm[:], wmat_ext[:])

            def emit_rows(bt, b0, e0, v_pad, Ct):
                # rows: t=0 plain; t>=1 threshold-masked, all independent
                rowgs = {}
                for t in range(T):
                    g = t // ROWS_PER_STORE
                    j = t % ROWS_PER_STORE
                    if g not in rowgs:
                        rowgs[g] = row_pool.tile(
                            [P, ROWS_PER_STORE, D], F16, tag="rowg",
                            name=f"rowg_{bt}_{g}",
                        )
                    if t == 0:
                        src_row = e0
                    else:
                        src_row = tmp_pool.tile([P, D], F32, tag="tmp")
                        # (e0 < v_t) * e0 ; v_t = t-th largest = v_pad[6+t]
                        nc.vector.scalar_tensor_tensor(
                            src_row[:], e0[:], v_pad[:, 6 + t : 7 + t],
                            e0[:], ALU.is_lt, ALU.mult,
                        )
                    nc.scalar.activation(
                        rowgs[g][:, j, :], src_row[:], AF.Copy, bias=0.0,
                        scale=Ct[:, t : t + 1],
                    )
                    if j == ROWS_PER_STORE - 1:
                        nc.sync.dma_start(
                            out_ext[b0 : b0 + P, t - j : t + 1, :],
                            rowgs[g][:],
                        )

            pending = None
            for bt in range(n_tiles):
                b0 = bt * P
                e0 = e_pool.tile([P, D], F32, tag="e")

                for h in range(2):
                    d0 = h * DH
                    megas = [None] * N_MEGA

                    def load_mega(m):
                        M = mega_pool.tile([P, MEGA_L, DH], F32, tag="mega")
                        nc.sync.dma_start(
                            M[:],
                            half_ext[h][
                                b0 : b0 + P,
                                m * MEGA_L : (m + 1) * MEGA_L,
                                :,
                            ],
                        )
                        megas[m] = M

                    # PE megas early (they release slots fastest); DVE
                    # chain megas (7,8,9) soon after so its chain overlaps
                    if general:
                        order = list(range(N_MEGA))
                    else:
                        order = [0, 1, 7, 8, 9, 2, 3, 4, 5, 6]
                    for m in order:
                        load_mega(m)

                    pt = ps_pool.tile([P, DH], F32, tag="p")

                    if general:
                        # all 50 l's through PE with diag(w_l) stationaries
                        for l in range(L):
                            nc.tensor.matmul(
                                pt[:],
                                wm[:, l, :],
                                megas[l // MEGA_L][:, l % MEGA_L, :],
                                start=(l == 0),
                                stop=(l == L - 1),
                            )
                    else:
                        # ---- DVE: partial = sum of l=28..49 ----
                        w1 = w1_pool.tile([P, MEGA_L, DH], F32, tag="w1")
                        nc.vector.tensor_tensor(
                            w1[:], megas[7][:], megas[8][:], ALU.add
                        )
                        nc.vector.tensor_tensor(
                            w1[:], w1[:], megas[9][:], ALU.add
                        )
                        partial = part_pool.tile([P, DH], F32, tag="part")
                        nc.vector.tensor_tensor(
                            partial[:], w1[:, 0, :], w1[:, 1, :], ALU.add
                        )
                        for j in range(2, MEGA_L):
                            nc.vector.tensor_tensor(
                                partial[:], partial[:], w1[:, j, :], ALU.add
                            )
                        for j in range(MEGA_L):
                            nc.vector.tensor_tensor(
                                partial[:], partial[:], megas[6][:, j, :],
                                ALU.add,
                            )
                        for j in range(L_PE % MEGA_L, MEGA_L):
                            nc.vector.tensor_tensor(
                                partial[:], partial[:],
                                megas[L_PE // MEGA_L][:, j, :], ALU.add,
                            )

                        # ---- PE: identity-accumulate l=0..L_PE-1 ----
                        for l in range(L_PE):
                            nc.tensor.matmul(
                                pt[:],
                                wm[:, 0, :],
                                megas[l // MEGA_L][:, l % MEGA_L, :],
                                start=(l == 0),
                                stop=(l == L_PE - 1),
                            )

                    # ---- e-half: exp(s_PE) * exp(s_DVE) ----
                    # (keeps the PE group independent of the DVE partial;
                    # exp of each part, then one DVE multiply)
                    nc.scalar.activation(
                        e0[:, d0 : d0 + DH], pt[:], AF.Exp, bias=0.0,
                        scale=1.0,
                    )
                    if not general:
                        pd = part_pool.tile([P, DH], F32, tag="pexp")
                        nc.scalar.activation(
                            pd[:], partial[:], AF.Exp, bias=0.0, scale=1.0
                        )
                        nc.vector.tensor_tensor(
                            e0[:, d0 : d0 + DH], e0[:, d0 : d0 + DH],
                            pd[:], ALU.mult,
                        )

                # ---- rows of the PREVIOUS tile: emitted here so their
                # store-issues queue behind this tile's load-issues on the
                # in-order sync sequencer (no load-stream stall) ----
                if pending is not None:
                    emit_rows(*pending)

                # ---- top-24 values + R = sum of the rest ----
                st = small_pool.tile([P, 104], F32, tag="stats")
                v_pad = st[:, 0:31]
                suf = st[:, 32:56]
                Dt = st[:, 56:76]
                Ct = st[:, 76:96]
                R = st[:, 96:97]
                nc.vector.memset(v_pad[:, 0:7], -1.0)
                va = v_pad[:, 7:15]
                vb = v_pad[:, 15:23]
                vc = v_pad[:, 23:31]
                u = tmp_pool.tile([P, D], F32, tag="tmp")
                nc.vector.max(va, e0[:])
                nc.vector.match_replace(u[:], va, e0[:], 0.0)
                nc.vector.max(vb, u[:])
                nc.vector.match_replace(u[:], vb, u[:], 0.0)
                nc.vector.max(vc, u[:])
                nc.vector.match_replace(u[:], vc, u[:], 0.0)
                nc.vector.tensor_reduce(
                    R, u[:], axis=mybir.AxisListType.X, op=ALU.add
                )

                # ---- D_t = suffix_sum(v_{t+1..24}) + R ;  C = 1/D ----
                v_rev = v_pad[:, 30:6:-1]
                nc.vector.tensor_tensor_scan(
                    suf, v_rev, v_rev, 0.0, ALU.add, ALU.bypass
                )
                nc.vector.tensor_scalar(
                    Dt, suf[:, 23:3:-1], R, None, ALU.add
                )
                nc.vector.reciprocal(Ct, Dt)
                pending = (bt, b0, e0, v_pad, Ct)

            emit_rows(*pending)

    nc.finalize()
    return nc


_GRAPH_CACHE = {}


def _get_graph(w):
    w = np.asarray(w, dtype=np.float32).reshape(-1)
    assert w.shape[0] == L
    if np.all(w == w[0]):
        # uniform: identity stationary, weight folded into the exp scale
        # (exp scale handled by scaling e? no — exp(scale*s): the scale is
        # baked as s is unweighted; but uniform_w == 1.0 for the harness.
        # For uniform_w != 1.0 we reuse the general path to stay exact.)
        if float(w[0]) == 1.0:
            key = "uniform"
            if key not in _GRAPH_CACHE:
                _GRAPH_CACHE[key] = build_graph(general=False)
            wmat = np.eye(P, dtype=np.float32).reshape(P, 1, P)
            return _GRAPH_CACHE[key], wmat
    key = "general"
    if key not in _GRAPH_CACHE:
        _GRAPH_CACHE[key] = build_graph(general=True)
    wmat = np.zeros((P, L, P), dtype=np.float32)
    for l in range(L):
        np.fill_diagonal(wmat[:, l, :], w[l])
    return _GRAPH_CACHE[key], wmat


def kernel(state, weight_matrix):
    state = np.ascontiguousarray(np.asarray(state, dtype=np.float32))
    w = np.asarray(weight_matrix, dtype=np.float32)
    assert state.shape == (B_FULL, L, D), state.shape

    nc, wmat = _get_graph(w)
    # pre-split the d-halves so device loads are contiguous (layout prep,
    # same class as the per-core sharding slices)
    state_lo = np.ascontiguousarray(state[:, :, :DH])
    state_hi = np.ascontiguousarray(state[:, :, DH:])
    in_maps = [
        {
            "state0": state_lo[i * B_CORE : (i + 1) * B_CORE],
            "state1": state_hi[i * B_CORE : (i + 1) * B_CORE],
            "wmat": wmat,
        }
        for i in range(N_CORES)
    ]
    res = run_bass_kernel_spmd(nc, in_maps, core_ids=list(range(N_CORES)))
    out = np.concatenate(
        [
            np.asarray(res.results[i]["out"], dtype=np.float32)
            for i in range(N_CORES)
        ],
        axis=0,
    )
    return out
